# revision 8
# baseline (speedup 1.0000x reference)
"""DeltaNet forward on 8 Trainium2 NeuronCores (Bass/Tile).

Problem (hardcoded): x[8,1024,1024]; Wq/Wk/Wv/Wo [1024,1024]; Wbeta [1024,4];
g_norm [256]. B=8, N=1024, D=1024, H=4, Dk=Dv=256.

Sharding: one batch per core (8 cores), weights replicated. fp16 operands,
fp32 PSUM accumulation throughout. Per core:
  1. beta + q,k projections (xT-stationary fp16 matmuls), l2-normalize per
     head, PE-transpose into interleaved QKT layout [d, (chunk,{K|Q},128)].
  2. One dense interleaved stream over chunks t (keeps the PE MAC-busy so
     HAM stays at full clock): v-projection for chunk t+2, delta-chain for
     chunk t+2, delta-scan for chunk t, o_n transposes for finished chunks,
     and output-projection column groups as their inputs complete.
     Delta rule per chunk/head:
       A   = b ⊙ tril(K K^T, -1)
       U   = (I-A)(I + A^2 + A^4) (b ⊙ (V - K S))    [folded Neumann doubling]
       O   = Q S + tril(Q K^T) U ;  S += K^T U
  3. Per-head RMSNorm (g_norm folded into Wo on host);
     out^T = Wo'^T @ o_n^T (Wo-stationary), host transposes back.
"""
import os
import sys
import types

import numpy as np

import concourse.bass as bass
import concourse.tile as tile
from concourse import bacc, mybir
from concourse import bass_utils

F32 = mybir.dt.float32
F16 = mybir.dt.float16
AF = mybir.ActivationFunctionType
ALU = mybir.AluOpType

B, N, D, H = 8, 1024, 1024, 4
DK = DV = D // H          # 256
C = 128                   # chunk length == token tile
NT = N // C               # 8 token tiles / chunks
KT = D // 128             # 8 contraction tiles
N_CORES = 8

_NC_CACHE = {}
LAST_RESULTS = None


def _install_ntff_hook():
    """Provide antenv.axon_hooks (missing in this image) so BASS_TRACE works."""
    try:
        import antenv
        if "antenv.axon_hooks" in sys.modules:
            return
        from trn_agent_boot.trn_boot import _ntff_profile_via_ctypes
        hook = _ntff_profile_via_ctypes("/opt/axon/libaxon_pjrt.so")
        mod = types.ModuleType("antenv.axon_hooks")
        mod._hook = hook
        mod.get_axon_ntff_profile_hook = lambda: mod._hook
        mod.set_axon_ntff_profile_hook = lambda h: setattr(mod, "_hook", h)
        sys.modules["antenv.axon_hooks"] = mod
        antenv.axon_hooks = mod
    except Exception:
        pass


if os.environ.get("BASS_TRACE"):
    _install_ntff_hook()


def _emit(nc, tc, dr, out_dr):
    """Emit the whole per-core program. dr: dict of input DRAM APs."""
    from contextlib import ExitStack

    with ExitStack() as ctx:
        cpool = ctx.enter_context(tc.tile_pool(name="consts", bufs=1))
        spool = ctx.enter_context(tc.tile_pool(name="smalls", bufs=4))
        ppool = ctx.enter_context(tc.tile_pool(name="persist", bufs=1))

        # constants
        maskA = cpool.tile([C, C], F32, name="maskA", tag="maskA")
        maskG = cpool.tile([C, C], F32, name="maskG", tag="maskG")
        idt = cpool.tile([C, C], F16, name="idt", tag="idt")
        eps_t = cpool.tile([128, 1], F32, name="eps_t", tag="eps_t")
        nc.gpsimd.memset(eps_t[:], 1e-5)
        nc.sync.dma_start(maskA[:], dr["maskA"][:])
        nc.sync.dma_start(maskG[:], dr["maskG"][:])
        nc.sync.dma_start(idt[:], dr["ident"][:])
        wb = []
        for kk in range(KT):
            t = cpool.tile([128, H], F16, name=f"wb{kk}", tag=f"wb{kk}")
            nc.sync.dma_start(t[:], dr["Wbeta"][kk * 128:(kk + 1) * 128, :])
            wb.append(t)

        xT = []
        for kk in range(KT):
            t = ppool.tile([128, N], F16, name=f"xTt{kk}", tag=f"xT{kk}")
            nc.sync.dma_start(t[:], dr["xT"][kk * 128:(kk + 1) * 128, :])
            xT.append(t)

        beta_sb = [cpool.tile([C, H], F32, name=f"beta{tt}", tag=f"beta{tt}")
                   for tt in range(NT)]

        qkt = [ppool.tile([128, 2 * N], F16, name=f"qkt{p}", tag=f"qkt{p}")
               for p in range(KT)]
        kn = [ppool.tile([128, D], F16, name=f"kn{tt}", tag=f"kn{tt}")
              for tt in range(NT)]
        vsb = [ppool.tile([128, D], F16, name=f"v{tt}", tag=f"v{tt}")
               for tt in range(NT)]
        o_n = [ppool.tile([128, D], F16, name=f"on{tt}", tag=f"on{tt}")
               for tt in range(NT)]
        onT = [ppool.tile([128, N], F16, name=f"ot{p}", tag=f"ot{p}")
               for p in range(KT)]
        S = [[ppool.tile([128, DV], F16, name=f"s{h}{di}", tag=f"s{h}{di}")
              for di in range(2)] for h in range(H)]
        wv = [[None] * KT for _ in range(2)]   # resident Wv tiles
        for n in range(2):
            for kk in range(KT):
                w = ppool.tile([128, 512], F16, name=f"wvt{n}{kk}", tag=f"wv{n}{kk}")
                nc.sync.dma_start(
                    w[:], dr["Wv"][kk * 128:(kk + 1) * 128, n * 512:(n + 1) * 512])
                wv[n][kk] = w

        def norm_head(ps, sub, dest_ap):
            """l2-normalize ps[:, sub*256:+256] into dest_ap (f16)."""
            sl = ps[:, sub * DK:(sub + 1) * DK]
            sq = spool.tile([128, DK], F32, name="sqscr", tag="sqscr")
            ss = spool.tile([128, 1], F32, name="ss", tag="ss")
            nc.scalar.activation(sq[:], sl, AF.Square, accum_out=ss[:])
            sr = spool.tile([128, 1], F32, name="sr", tag="sr")
            nc.scalar.activation(sr[:], ss[:], AF.Sqrt)
            ri = spool.tile([128, 1], F32, name="ri", tag="ri")
            nc.vector.reciprocal(ri[:], sr[:])
            nc.vector.tensor_scalar_mul(dest_ap, sl, ri[:])

        # ---------------- phase A: beta, q,k projections + transposes --------
        with ExitStack() as p1:
            wpool = p1.enter_context(tc.tile_pool(name="wts", bufs=1))
            qnpool = p1.enter_context(tc.tile_pool(name="qn", bufs=4))
            pjpsA = p1.enter_context(tc.tile_pool(name="pjpsA", bufs=3, space="PSUM"))
            btps = p1.enter_context(tc.tile_pool(name="btps", bufs=1, space="PSUM"))
            trps = p1.enter_context(tc.tile_pool(name="trps", bufs=4, space="PSUM"))

            for tt in range(NT):
                bp = btps.tile([C, H], F32, name="btp", tag="btp")
                for kk in range(KT):
                    nc.tensor.matmul(bp[:], xT[kk][:, tt * 128:(tt + 1) * 128],
                                     wb[kk][:], start=(kk == 0), stop=(kk == KT - 1))
                nc.scalar.activation(beta_sb[tt][:], bp[:], AF.Sigmoid)

            def emit_transposes(which, tt, n, qn_tile):
                """PE-transpose normalized q/k block (tt, n) into QKT."""
                for sub in range(2):
                    h = 2 * n + sub
                    for di in range(2):
                        p = h * 2 + di
                        if which == "q":
                            src = qn_tile[:, sub * DK + di * 128:
                                          sub * DK + (di + 1) * 128]
                            off = tt * 256 + 128
                        else:
                            src = kn[tt][:, n * 512 + sub * DK + di * 128:
                                          n * 512 + sub * DK + (di + 1) * 128]
                            off = tt * 256
                        tp = trps.tile([128, 128], F16, name="tr", tag="tr")
                        nc.tensor.transpose(tp[:], src, idt[:])
                        nc.vector.tensor_copy(qkt[p][:, off:off + 128], tp[:])

            for proj, wdr in (("q", dr["Wq"]), ("k", dr["Wk"])):
                pending = []
                for n in range(2):
                    wt = []
                    for kk in range(KT):
                        w = wpool.tile([128, 512], F16, name=f"w{kk}", tag=f"w{kk}")
                        nc.sync.dma_start(
                            w[:], wdr[kk * 128:(kk + 1) * 128, n * 512:(n + 1) * 512])
                        wt.append(w)
                    for tt in range(NT):
                        ps = pjpsA.tile([128, 512], F32, name="pj", tag="pj")
                        for kk in range(KT):
                            nc.tensor.matmul(
                                ps[:], xT[kk][:, tt * 128:(tt + 1) * 128], wt[kk][:],
                                start=(kk == 0), stop=(kk == KT - 1))
                        if proj == "q":
                            qn_tile = qnpool.tile([128, 512], F16, name="qn", tag="qn")
                            for sub in range(2):
                                norm_head(ps, sub, qn_tile[:, sub * DK:(sub + 1) * DK])
                            pending.append(("q", tt, n, qn_tile))
                        else:
                            for sub in range(2):
                                norm_head(ps, sub, kn[tt][:, n * 512 + sub * DK:
                                                          n * 512 + (sub + 1) * DK])
                            pending.append(("k", tt, n, None))
                        if len(pending) >= 3:
                            emit_transposes(*pending.pop(0))
                for args in pending:
                    emit_transposes(*args)

        # -------- phase B: one dense stream: v-proj / chains / scans / out ----
        with ExitStack() as p2:
            chp = p2.enter_context(tc.tile_pool(name="chain", bufs=3))
            tmpp = p2.enter_context(tc.tile_pool(name="chtmp", bufs=2))
            upool = p2.enter_context(tc.tile_pool(name="u", bufs=6))
            wopool = p2.enter_context(tc.tile_pool(name="wo", bufs=2))
            outsb = p2.enter_context(tc.tile_pool(name="outsb", bufs=3))
            pjps = p2.enter_context(tc.tile_pool(name="pjps", bufs=1, space="PSUM"))
            dps = p2.enter_context(tc.tile_pool(name="dps", bufs=3, space="PSUM"))
            ops = p2.enter_context(tc.tile_pool(name="ops", bufs=1, space="PSUM"))
            p16 = p2.enter_context(tc.tile_pool(name="p16", bufs=2, space="PSUM"))
            cmps = p2.enter_context(tc.tile_pool(name="cmps", bufs=1, space="PSUM"))

            chain_out = {}

            def emit_vproj(t):
                for n in range(2):
                    ps = pjps.tile([128, 512], F32, name="vpj", tag="pj")
                    for kk in range(KT):
                        nc.tensor.matmul(ps[:], xT[kk][:, t * 128:(t + 1) * 128],
                                         wv[n][kk][:], start=(kk == 0),
                                         stop=(kk == KT - 1))
                    nc.vector.tensor_copy(vsb[t][:, n * 512:(n + 1) * 512], ps[:])

            def emit_chain(t, h):
                b_col = beta_sb[t][:, h:h + 1]
                g = dps.tile([128, 256], F32, name="g", tag="d")
                for di in range(2):
                    p = h * 2 + di
                    nc.tensor.matmul(g[:], qkt[p][:, t * 256:t * 256 + 128],
                                     qkt[p][:, t * 256:t * 256 + 256],
                                     start=(di == 0), stop=(di == 1))
                A = tmpp.tile([128, 128], F16, name="A", tag="At")
                nc.vector.tensor_tensor(out=A[:], in0=g[:, 0:128], in1=maskA[:],
                                        op=ALU.mult)
                nc.scalar.activation(A[:], A[:], AF.Copy, bias=0.0, scale=b_col)
                Gm = chp.tile([128, 128], F16, name="Gm", tag=f"Gm{h}")
                nc.vector.tensor_tensor(out=Gm[:], in0=g[:, 128:256], in1=maskG[:],
                                        op=ALU.mult)
                Cp = p16.tile([128, 128], F16, name="Cp", tag="ct")
                nc.tensor.transpose(Cp[:], A[:], idt[:])
                Csb = chp.tile([128, 128], F16, name="Csb", tag=f"C{h}")
                nc.scalar.copy(Csb[:], Cp[:])
                C2p = cmps.tile([128, 128], F32, name="C2p", tag="cm")
                nc.tensor.matmul(C2p[:], A[:], Csb[:], start=True, stop=True)
                C2sb = tmpp.tile([128, 128], F16, name="C2sb", tag="C2t")
                nc.scalar.copy(C2sb[:], C2p[:])
                A2p = p16.tile([128, 128], F16, name="A2p", tag="ct")
                nc.tensor.transpose(A2p[:], C2sb[:], idt[:])
                A2 = tmpp.tile([128, 128], F16, name="A2", tag="A2t")
                nc.vector.tensor_copy(A2[:], A2p[:])
                C4p = cmps.tile([128, 128], F32, name="C4p", tag="cm")
                nc.tensor.matmul(C4p[:], A2[:], C2sb[:], start=True, stop=True)
                # W24 = C^2 + C^4  (folded (I+A^2)(I+A^4) ~= I + A^2 + A^4)
                W24 = chp.tile([128, 128], F16, name="W24", tag=f"W24{h}")
                nc.vector.tensor_tensor(out=W24[:], in0=C4p[:], in1=C2sb[:],
                                        op=ALU.add)
                chain_out[(t, h)] = (Csb, W24, Gm)

            def emit_scan(t, h):
                Csb, W24, Gm = chain_out.pop((t, h))
                b_col = beta_sb[t][:, h:h + 1]
                vslice = vsb[t][:, h * DV:(h + 1) * DV]
                U0 = upool.tile([128, DV], F16, name="U0", tag="u")
                if t == 0:
                    nc.vector.tensor_scalar_mul(U0[:], vslice, b_col)
                else:
                    kc = dps.tile([128, DV], F32, name="kc", tag="d")
                    for di in range(2):
                        p = h * 2 + di
                        nc.tensor.matmul(kc[:], qkt[p][:, t * 256:t * 256 + 128],
                                         S[h][di][:], start=(di == 0), stop=(di == 1))
                    U0m = upool.tile([128, DV], F16, name="U0m", tag="u")
                    nc.vector.tensor_tensor(out=U0m[:], in0=vslice, in1=kc[:],
                                            op=ALU.subtract)
                    nc.vector.tensor_scalar_mul(U0[:], U0m[:], b_col)
                # U = (I - A)(I + W24) U0
                Y1 = dps.tile([128, DV], F32, name="Y1", tag="d")
                nc.tensor.matmul(Y1[:], W24[:], U0[:], start=True, stop=True)
                U1 = upool.tile([128, DV], F16, name="U1", tag="u")
                nc.vector.tensor_tensor(out=U1[:], in0=U0[:], in1=Y1[:], op=ALU.add)
                Y3 = dps.tile([128, DV], F32, name="Y3", tag="d")
                nc.tensor.matmul(Y3[:], Csb[:], U1[:], start=True, stop=True)
                U3 = upool.tile([128, DV], F16, name="U3", tag="u")
                nc.vector.tensor_tensor(out=U3[:], in0=U1[:], in1=Y3[:],
                                        op=ALU.subtract)
                # O = Q S + Gm^T U
                O = ops.tile([128, DV], F32, name="O", tag="o")
                if t > 0:
                    for di in range(2):
                        p = h * 2 + di
                        nc.tensor.matmul(O[:], qkt[p][:, t * 256 + 128:t * 256 + 256],
                                         S[h][di][:], start=(di == 0), stop=False)
                nc.tensor.matmul(O[:], Gm[:], U3[:], start=(t == 0), stop=True)
                # RMS norm over DV (g folded into Wo on host)
                sq = spool.tile([128, DV], F32, name="sq", tag="sqscr")
                ss = spool.tile([128, 1], F32, name="ssr", tag="ss")
                nc.scalar.activation(sq[:], O[:], AF.Square, accum_out=ss[:])
                sr = spool.tile([128, 1], F32, name="srr", tag="sr")
                nc.scalar.activation(sr[:], ss[:], AF.Sqrt, bias=eps_t[:],
                                     scale=1.0 / DV)
                ri = spool.tile([128, 1], F32, name="rir", tag="ri")
                nc.vector.reciprocal(ri[:], sr[:])
                nc.vector.tensor_scalar_mul(o_n[t][:, h * DV:(h + 1) * DV], O[:],
                                            ri[:])
                # S += K^T U
                for di in range(2):
                    dS = dps.tile([128, DV], F32, name="dS", tag="d")
                    nc.tensor.matmul(
                        dS[:], kn[t][:, h * DK + di * 128:h * DK + (di + 1) * 128],
                        U3[:], start=True, stop=True)
                    if t == 0:
                        nc.vector.tensor_copy(S[h][di][:], dS[:])
                    else:
                        nc.vector.tensor_tensor(out=S[h][di][:], in0=S[h][di][:],
                                                in1=dS[:], op=ALU.add)

            def emit_ontr(t):
                for p in range(KT):
                    tp = p16.tile([128, 128], F16, name="ftp", tag="ct")
                    nc.tensor.transpose(tp[:], o_n[t][:, p * 128:(p + 1) * 128],
                                        idt[:])
                    nc.vector.tensor_copy(onT[p][:, t * 128:(t + 1) * 128], tp[:])

            def emit_final(oc, nh):
                wt = []
                for p in range(KT):
                    w = wopool.tile([128, 128], F16, name=f"wog{p}", tag=f"wo{p}")
                    nc.sync.dma_start(
                        w[:], dr["Wog"][p * 128:(p + 1) * 128,
                                        oc * 128:(oc + 1) * 128])
                    wt.append(w)
                ps = pjps.tile([128, 512], F32, name="fpp", tag="pj")
                for p in range(KT):
                    nc.tensor.matmul(ps[:], wt[p][:],
                                     onT[p][:, nh * 512:(nh + 1) * 512],
                                     start=(p == 0), stop=(p == KT - 1))
                ob = outsb.tile([128, 512], F32, name="ob", tag="ob")
                nc.scalar.copy(ob[:], ps[:])
                nc.sync.dma_start(
                    out_dr[oc * 128:(oc + 1) * 128, nh * 512:(nh + 1) * 512], ob[:])

            emit_vproj(0)
            emit_vproj(1)
            for h in range(H):
                emit_chain(0, h)
            for h in range(H):
                emit_chain(1, h)
            for t in range(NT):
                if t + 2 < NT:
                    emit_vproj(t + 2)
                for h in range(H):
                    if t + 2 < NT:
                        emit_chain(t + 2, h)
                    emit_scan(t, h)
                emit_ontr(t)
                if t >= 4:
                    emit_final(2 * (t - 4), 0)
                    emit_final(2 * (t - 4) + 1, 0)
            for oc in range(KT):
                if oc >= 6:
                    emit_final(oc, 0)
                emit_final(oc, 1)


def build():
    if "nc" in _NC_CACHE:
        return _NC_CACHE["nc"]
    nc = bacc.Bacc("TRN2", target_bir_lowering=False, debug=False)
    dr = {
        "xT": nc.dram_tensor("xT", [D, N], F16, kind="ExternalInput").ap(),
        "Wq": nc.dram_tensor("Wq", [D, D], F16, kind="ExternalInput").ap(),
        "Wk": nc.dram_tensor("Wk", [D, D], F16, kind="ExternalInput").ap(),
        "Wv": nc.dram_tensor("Wv", [D, D], F16, kind="ExternalInput").ap(),
        "Wbeta": nc.dram_tensor("Wbeta", [D, H], F16, kind="ExternalInput").ap(),
        "Wog": nc.dram_tensor("Wog", [D, D], F16, kind="ExternalInput").ap(),
        "maskA": nc.dram_tensor("maskA", [C, C], F32, kind="ExternalInput").ap(),
        "maskG": nc.dram_tensor("maskG", [C, C], F32, kind="ExternalInput").ap(),
        "ident": nc.dram_tensor("ident", [C, C], F16, kind="ExternalInput").ap(),
    }
    out_dr = nc.dram_tensor("out", [D, N], F32, kind="ExternalOutput").ap()
    with tile.TileContext(nc) as tc:
        _emit(nc, tc, dr, out_dr)
    nc.compile()
    _NC_CACHE["nc"] = nc
    return nc


def kernel(x, Wq, Wk, Wv, Wbeta, Wo, g_norm):
    global LAST_RESULTS
    x = np.asarray(x, np.float32)
    f16c = lambda a: np.ascontiguousarray(np.asarray(a, np.float32)).astype(np.float16)
    Wq16, Wk16, Wv16, Wb16 = f16c(Wq), f16c(Wk), f16c(Wv), f16c(Wbeta)
    Wo = np.asarray(Wo, np.float32)
    g_norm = np.asarray(g_norm, np.float32)

    Wog = (np.tile(g_norm, H)[:, None] * Wo).astype(np.float16)
    maskA = np.tril(np.ones((C, C), np.float32), -1)
    maskG = np.triu(np.ones((C, C), np.float32), 0)  # [s,t] keep s<=t
    ident = np.eye(C, dtype=np.float16)

    nc = build()
    in_maps = []
    for b in range(B):
        in_maps.append({
            "xT": np.ascontiguousarray(x[b].T).astype(np.float16),
            "Wq": Wq16, "Wk": Wk16, "Wv": Wv16, "Wbeta": Wb16, "Wog": Wog,
            "maskA": maskA, "maskG": maskG, "ident": ident,
        })
    res = bass_utils.run_bass_kernel_spmd(nc, in_maps, core_ids=list(range(N_CORES)))
    LAST_RESULTS = res
    return np.stack([res.results[b]["out"].T for b in range(B)], axis=0)
